# revision 49
# baseline (speedup 1.0000x reference)
"""Trainium2 Bass kernel for nn_BasicDeconvolutionBlock.

Reference computation:
    gathered = feats[in_map]                         # [K, M, Cin]
    contrib  = einsum('kmc,kcd->kmd', gathered, W)   # [K, M, Cout]
    out      = zeros([n_out, Cout]).at[out_map].add(contrib)
    y        = relu(batchnorm(out))                  # batch stats over n_out rows

Strategy v2 (8 NeuronCores, SPMD, matmul-scatter, balanced routing):
  - Output rows are assigned to 1632 bins (204 per core, <=128 rows each)
    by a host-side balancer (shave + reinsert + swap) so that EVERY
    (bin, k-pair) group has <= 64 pairs on every core.  Group caps are
    then uniformly 64 (or 32), eliminating the 32-align/96-bump padding
    of v1: S/core = ~183k slots vs 227k.
  - k-PAIR PACKING: offsets (2j, 2j+1) share one GEMM with stacked
    weights [W_2j; W_2j+1] ([128, 64]); a pair's feats occupy channel
    half k%2, zeros in the other half.
  - Groups ordered caps-64-first within each block so every group is a
    single legal matmul run (PSUM partition bases 0/64 for 64-wide,
    0/32/64/96 for 32-wide).  Blocks pad to 128-slot tiles.
  - Blocks hold <=128 output rows -> scatter one-hot P is [128 x 128]
    and each tile needs ONE scatter matmul (v1: [128 x 256], two).
  - P tiles via tensor_scalar(is_equal) vs an iota constant, split
    ~70/30 across DVE and Pool (gpsimd).  Pad slots (ridx=-1) give zero
    columns.
  - Host pre-gathers feats into channel-major ftab[128, S] bf16,
    streamed sequentially in large segments.
  - Software pipelining: batch g's scatters issue after batch g+1's
    mains (hides the contrib PSUM->SBUF copy); acc copies lag one
    iteration and BN chains two, so no engine waits on another's
    just-issued work (engine queues are in-order).
  - BN: every acc block carries a constant ones-column (65 wide); one
    chained self-Gram matmul per block yields Sum(x^2) on the diagonal
    and Sum(x) in row 64.  [2,64] partials AllGathered then reduced
    on-chip; normalize (mult/add on DVE+Pool, ReLU on Act) chunked and
    overlapped with the bf16 y writeback.
"""

import numpy as np

import sys

sys.path.insert(0, "/opt/trn_rl_repo")

import ml_dtypes  # noqa: E402

from concourse import bacc, mybir  # noqa: E402
import concourse.tile as tile  # noqa: E402

BN_EPS = 1e-5
F32 = mybir.dt.float32
BF16 = mybir.dt.bfloat16

NBINS_PER_CORE = 204
CAP = 64
ROWCAP = 128


def _roundup(x, m):
    return (x + m - 1) // m * m


def _balance_rows(deg, n_out, nbins, rng):
    """Assign rows to bins s.t. every (bin, j) count <= CAP and rows <= ROWCAP.

    Returns assign [n_out] -> bin id.
    """
    J = deg.shape[1]
    assign = np.empty(n_out, np.int32)
    perm = rng.permutation(n_out)
    assign[perm] = np.arange(n_out) % nbins
    binj = np.zeros((nbins, J), np.int64)
    for j in range(J):
        np.add.at(binj[:, j], assign, deg[:, j])
    binrows = np.bincount(assign, minlength=nbins).astype(np.int64)

    TRIM = CAP - 6
    pool = []
    rows_by_bin = [list(np.nonzero(assign == b)[0]) for b in range(nbins)]
    for b in range(nbins):
        guard = 0
        while (binj[b] > TRIM).any() and guard < 400:
            guard += 1
            jbad = int(np.argmax(binj[b]))
            cand = [r for r in rows_by_bin[b] if deg[r, jbad] > 0]
            if not cand:
                break
            r = max(cand, key=lambda r: deg[r, jbad])
            rows_by_bin[b].remove(r)
            pool.append(r)
            binj[b] -= deg[r]
            binrows[b] -= 1
            assign[r] = -1

    pool.sort(key=lambda r: -deg[r].max())
    hard = []
    for r in pool:
        v = deg[r]
        js = np.nonzero(v)[0]
        ok = binrows < ROWCAP
        for j in js:
            ok &= binj[:, j] + v[j] <= CAP - 1
        cand = np.nonzero(ok)[0]
        if len(cand) == 0:
            hard.append(r)
            continue
        t = int(cand[np.argmin(binj[cand].max(1))])
        assign[r] = t
        binj[t] += v
        binrows[t] += 1
    for r in hard:
        v = deg[r]
        js = np.nonzero(v)[0]
        ok = binrows < ROWCAP
        for j in js:
            ok &= binj[:, j] + v[j] <= CAP
        cand = np.nonzero(ok)[0]
        if len(cand) == 0:
            score = np.maximum(binj[:, js] + v[js][None, :] - CAP, 0).sum(1) + \
                np.where(binrows >= ROWCAP, 10000, 0)
            t = int(np.argmin(score))
        else:
            t = int(cand[np.argmin(binj[cand].max(1))])
        assign[r] = t
        binj[t] += v
        binrows[t] += 1

    # swap repair for any residual over-cap groups
    for _ in range(6):
        over = np.argwhere(binj > CAP)
        if len(over) == 0:
            break
        for b, j in over:
            guard = 0
            while binj[b, j] > CAP and guard < 20:
                guard += 1
                rows_b = np.nonzero((assign == b) & (deg[:, j] > 0))[0]
                r = rows_b[np.argmax(deg[rows_b, j])]
                v = deg[r]
                js = np.nonzero(v)[0]
                ok = binrows < ROWCAP
                for jj in js:
                    ok &= binj[:, jj] + v[jj] <= CAP
                ok[b] = False
                cand = np.nonzero(ok)[0]
                if len(cand):
                    t = int(cand[np.argmin(binj[cand].max(1))])
                    assign[r] = t
                    binj[t] += v
                    binrows[t] += 1
                    binj[b] -= v
                    binrows[b] -= 1
                    continue
                done = False
                for t in np.argsort(binj[:, j])[:60]:
                    if t == b:
                        continue
                    rows_t = np.nonzero(assign == t)[0]
                    for r2 in rows_t[np.argsort(deg[rows_t].max(1))[:40]]:
                        v2 = deg[r2]
                        nb = binj[b] - v + v2
                        nt = binj[t] - v2 + v
                        if (nb <= CAP).all() and (nt <= CAP).all():
                            assign[r] = t
                            assign[r2] = b
                            binj[b] = nb
                            binj[t] = nt
                            done = True
                            break
                    if done:
                        break
                if not done:
                    break
    return assign, binj


def _route(in_map, out_map, n_out, n_cores):
    """Host routing v2. Returns (plan, per-core slot tables, row_order)."""
    K, M = in_map.shape
    J = (K + 1) // 2
    nbins = NBINS_PER_CORE * n_cores
    NBLK = NBINS_PER_CORE

    k_idx = np.repeat(np.arange(K, dtype=np.int64), M)
    in_flat = in_map.ravel().astype(np.int64)
    out_flat = out_map.ravel().astype(np.int64)
    j_idx = k_idx // 2
    half = (k_idx % 2).astype(np.int64)

    deg = np.zeros((n_out, J), np.int64)
    np.add.at(deg, (out_flat, j_idx), 1)

    rng = np.random.default_rng(0)
    assign, binj = _balance_rows(deg, n_out, nbins, rng)

    # bins -> (core, position): sort by total load, deal round-robin
    ordv = np.argsort(-binj.sum(1), kind="stable")
    core_of_bin = np.empty(nbins, np.int32)
    pos_of_bin = np.empty(nbins, np.int32)
    for i, b in enumerate(ordv):
        core_of_bin[b] = i % n_cores
        pos_of_bin[b] = i // n_cores

    # caps[pos, j] = roundup32(max over cores of binj)
    posmax = np.zeros((NBLK, J), np.int64)
    np.maximum.at(posmax, pos_of_bin, binj)
    caps = ((posmax + 31) // 32) * 32  # 0 / 32 / 64
    # order groups within a block: 64-caps first, then 32s (single legal runs)
    jorder = [np.argsort(-caps[p], kind="stable") for p in range(NBLK)]

    # block slot layout
    goff = np.zeros((NBLK, J), np.int64)     # slot offset of group (pos, j)
    btot = np.zeros(NBLK, np.int64)
    for p in range(NBLK):
        s = 0
        for j in jorder[p]:
            goff[p, j] = s
            s += caps[p, j]
        btot[p] = _roundup(s, 128)
    bstart = np.concatenate([[0], np.cumsum(btot)])
    S = int(bstart[-1])
    ntiles = S // 128
    goff += bstart[:-1][:, None]

    # tiles: per tile (block, [(j, c0, c1) runs]); caps 64/32 at 32-mult
    # offsets never need splitting (bases 0/32/64/96 all legal for w<=32,
    # 0/64 for w<=64)
    tile_block = np.searchsorted(bstart, np.arange(ntiles) * 128, side="right") - 1
    runs_per_tile = [[] for _ in range(ntiles)]
    for p in range(NBLK):
        for j in range(J):
            c = int(caps[p, j])
            if c == 0:
                continue
            g0 = int(goff[p, j])
            # split at tile boundaries, then at legal PSUM quadrant bases
            # (0: any width, 32: <=32, 64: <=64, 96: <=32)
            pos = g0
            while pos < g0 + c:
                t = pos // 128
                c0 = pos - t * 128
                c1 = min(g0 + c - t * 128, 128)
                while c0 < c1:
                    if c0 == 0:
                        e = c1
                    elif c0 == 32:
                        e = min(c1, 64)
                    elif c0 == 64:
                        e = min(c1, 128)
                    elif c0 == 96:
                        e = c1
                    else:
                        raise AssertionError((p, j, c0, c1))
                    runs_per_tile[t].append((int(j), c0, e))
                    c0 = e
                pos = t * 128 + c1
    tiles = [(int(tile_block[t]), runs_per_tile[t]) for t in range(ntiles)]

    # per-core slot tables + row ordering
    # row index within bin: sorted ascending global row id
    order_rows = np.argsort(assign, kind="stable")
    rows_sorted = order_rows[assign[order_rows] >= 0]
    # rows grouped by bin
    bin_of = assign[rows_sorted]
    starts = np.searchsorted(bin_of, np.arange(nbins))
    ends = np.searchsorted(bin_of, np.arange(nbins) + 1)
    ridx_of_row = np.full(n_out, -1, np.int64)
    for b in range(nbins):
        rs = rows_sorted[starts[b]:ends[b]]
        ridx_of_row[rs] = np.arange(len(rs))

    # row_order[core, pos] = global rows (for host unpack)
    row_order = np.full((n_cores, NBLK, ROWCAP), -1, np.int64)
    for b in range(nbins):
        rs = rows_sorted[starts[b]:ends[b]]
        row_order[core_of_bin[b], pos_of_bin[b], :len(rs)] = rs

    bin_of_pair = assign[out_flat]
    core_of_pair = core_of_bin[bin_of_pair]
    pos_of_pair = pos_of_bin[bin_of_pair]
    rix_of_pair = ridx_of_row[out_flat]
    assert rix_of_pair.max() < ROWCAP and rix_of_pair.min() >= 0

    per_core = []
    for c in range(n_cores):
        sel = np.nonzero(core_of_pair == c)[0]
        o = np.lexsort((in_flat[sel], j_idx[sel], pos_of_pair[sel]))
        sel = sel[o]
        cb, cj = pos_of_pair[sel], j_idx[sel]
        key = cb * J + cj
        n = len(key)
        first = np.ones(n, dtype=bool)
        first[1:] = key[1:] != key[:-1]
        grp_start = np.maximum.accumulate(np.where(first, np.arange(n), 0))
        pos = np.arange(n) - grp_start
        slots = goff[cb, cj] + pos
        assert (pos < caps[cb, cj]).all()

        feat_row = np.full(S, -1, np.int64)
        s_half = np.zeros(S, np.int64)
        ridx = np.full(S, -1.0, np.float32)
        feat_row[slots] = in_flat[sel]
        s_half[slots] = half[sel]
        ridx[slots] = rix_of_pair[sel].astype(np.float32)
        per_core.append(dict(feat_row=feat_row, half=s_half, ridx=ridx))

    plan = dict(
        S=S, ntiles=ntiles, NBLK=NBLK, J=J,
        tiles=tiles, bstart=bstart.tolist(),
        row_order=row_order,
        # test.py compat
        R=1, total_slots=S, segments=[None] * ((S + 4095) // 4096),
        rows_per_core=n_out // n_cores, acc_rows=NBLK * 128,
    )
    return plan, per_core


def _build(plan, n_out, n_cores):
    nc = bacc.Bacc("TRN2", target_bir_lowering=False, debug=False)

    S, ntiles, NBLK, J = plan["S"], plan["ntiles"], plan["NBLK"], plan["J"]
    tiles = plan["tiles"]
    bstart = plan["bstart"]
    SEG = 2048   # slots per ftab DMA
    GRP = 16     # tiles per contrib PSUM batch ([128, 1024] f32 = 2 banks)

    ftab = nc.dram_tensor("ftab", [128, S], BF16, kind="ExternalInput")
    wt = nc.dram_tensor("wt", [128, J * 64], BF16, kind="ExternalInput")
    ridx = nc.dram_tensor("ridx", [128, ntiles], F32, kind="ExternalInput")
    cio = nc.dram_tensor("cio", [128, 128], BF16, kind="ExternalInput")
    pidx = nc.dram_tensor("pidx", [128, 1], F32, kind="ExternalInput")
    gb_t = nc.dram_tensor("gb", [2, 64], F32, kind="ExternalInput")
    cc_in = nc.dram_tensor("cc_in", [2, 64], F32)
    cc_out = nc.dram_tensor("cc_out", [2 * n_cores, 64], F32, addr_space="Shared")
    y = nc.dram_tensor("y", [128, NBLK * 65], BF16, kind="ExternalOutput")

    nseg = (S + SEG - 1) // SEG
    ngrp = (ntiles + GRP - 1) // GRP

    with tile.TileContext(nc) as tc:
      with tc.tile_pool(name="acc", bufs=1) as apool, \
           tc.tile_pool(name="bnch", bufs=1, space="PSUM") as bnchpool:
        with (
            tc.tile_pool(name="const", bufs=1) as cpool,
            tc.tile_pool(name="g", bufs=8) as gpool,
            tc.tile_pool(name="cp", bufs=5) as csbpool,
            tc.tile_pool(name="pg", bufs=5) as ppool,
            tc.tile_pool(name="cps", bufs=2, space="PSUM") as cpspool,
            tc.tile_pool(name="aps", bufs=3, space="PSUM") as apspool,
        ):
            w_sb = cpool.tile([128, J * 64], BF16, tag="w")
            nc.sync.dma_start(out=w_sb[:, :], in_=wt[:, :])
            g_tiles = [None] * nseg
            g0 = gpool.tile([128, SEG], BF16, tag="g")
            nc.sync.dma_start(out=g0[:, :SEG], in_=ftab[:, 0:SEG])
            g_tiles[0] = (g0, 0)
            cio_sb = apool.tile([128, 128], BF16, tag="cio")
            nc.sync.dma_start(out=cio_sb[:, :], in_=cio[:, :])
            ridx_sb = cpool.tile([128, ntiles], F32, tag="ridx")
            for rc in range(4):
                r0 = rc * ((ntiles + 3) // 4)
                r1 = min(r0 + (ntiles + 3) // 4, ntiles)
                if r0 < r1:
                    nc.sync.dma_start(out=ridx_sb[:, r0:r1],
                                      in_=ridx[:, r0:r1])
            pidx_sb = apool.tile([128, 1], F32, tag="pidx")
            nc.sync.dma_start(out=pidx_sb[:, :], in_=pidx[:, :])
            warm = cpool.tile([1, 1], F32, tag="warm")
            nc.vector.memset(warm[:, :], 1.0)
            nc.scalar.activation(
                out=warm[:, :], in_=warm[:, :],
                func=mybir.ActivationFunctionType.Sqrt)
            acc_sb = apool.tile([128, NBLK, 65], BF16, tag="acc")
            # col 64 of every block is a constant 1.0: the per-block
            # self-Gram matmul then yields Sum(x^2) on its diagonal AND
            # Sum(x) in row 64, one chained matmul per block
            nc.vector.memset(acc_sb[:, :, 64:65], 1.0)
            gs_ps = bnchpool.tile([128, 512], F32, tag="gs")

            contrib = [None] * ngrp
            pslab = [None] * ngrp
            acc_ps = None
            pending_copies = []
            pending_chains = []
            rr = 0
            blk_copy_rr = 0

            def _bn_chain(blk):
                # chained BN stats: [gram | Sum(x)] in one matmul
                nc.tensor.matmul(
                    out=gs_ps[0:65, 0:65],
                    lhsT=acc_sb[:, blk, :],
                    rhs=acc_sb[:, blk, :],
                    start=(blk == 0), stop=(blk == NBLK - 1),
                    skip_group_check=True)

            # software-pipelined: batch g's scatters are issued AFTER batch
            # g+1's mains, so the contrib PSUM->SBUF copy latency hides
            # behind the next batch's PE work instead of stalling scatters
            for grp in range(ngrp + 1):
                if grp < ngrp:
                    t0 = grp * GRP
                    t1 = min(t0 + GRP, ntiles)
                    cps = cpspool.tile([128, GRP * 64], F32, tag="cps")
                    P = ppool.tile([128, GRP, 128], BF16, tag="P")
                    csb_t = csbpool.tile([128, GRP, 64], BF16, tag="csb")
                    contrib[grp] = csb_t
                    half = t0 + (t1 - t0 + 1) // 2
                    for t in range(t0, t1):
                        seg = t * 128 // SEG
                        if g_tiles[seg] is None:
                            s0 = seg * SEG
                            s1 = min(s0 + SEG, S)
                            g = gpool.tile([128, SEG], BF16, tag="g")
                            nc.sync.dma_start(out=g[:, : s1 - s0],
                                              in_=ftab[:, s0:s1])
                            g_tiles[seg] = (g, s0)
                        g, s0 = g_tiles[seg]
                        toff = t * 128 - s0
                        blk_t, runs = tiles[t]
                        for (j, c0, c1) in runs:
                            nc.tensor.matmul(
                                out=cps[c0:c1, (t - t0) * 64:(t - t0 + 1) * 64],
                                lhsT=g[:, toff + c0: toff + c1],
                                rhs=w_sb[:, j * 64:(j + 1) * 64],
                                start=True, stop=True, skip_group_check=True,
                            )
                        # P tile: 1[cio[r] == ridx[s]]; pads -> zero cols
                        eng = nc.vector if rr % 25 < 17 else nc.gpsimd
                        rr += 1
                        eng.tensor_scalar(
                            out=P[:, t - t0, :], in0=cio_sb[:, :],
                            scalar1=ridx_sb[:, t:t + 1], scalar2=None,
                            op0=mybir.AluOpType.is_equal,
                        )
                    nc.scalar.activation(
                        out=csb_t[:, : t1 - t0, :],
                        in_=cps[:, : (t1 - t0) * 64],
                        func=mybir.ActivationFunctionType.Copy,
                    )
                    pslab[grp] = P

                # acc copies for blocks completed LAST iteration (scatter
                # deps a full batch old -> DVE/Act don't stall on PE), then
                # BN chains for blocks copied the iteration BEFORE (copy
                # deps a full batch old -> PE doesn't stall on DVE/Act)
                for blk_t, aps_h in pending_copies:
                    blk_copy_rr += 1
                    if blk_copy_rr % 5 < 2:
                        nc.vector.tensor_copy(
                            out=acc_sb[:, blk_t, 0:64], in_=aps_h[:, 0:64])
                    else:
                        nc.scalar.activation(
                            out=acc_sb[:, blk_t, 0:64], in_=aps_h[:, 0:64],
                            func=mybir.ActivationFunctionType.Copy)
                for blk_t in pending_chains:
                    _bn_chain(blk_t)
                pending_chains = [b for b, _ in pending_copies]
                pending_copies = []

                if grp == 0:
                    continue
                # scatter the PREVIOUS batch's tiles (one matmul each, B=128)
                sg = grp - 1
                for t in range(sg * GRP, min(sg * GRP + GRP, ntiles)):
                    blk_t, _ = tiles[t]
                    first = (t * 128 == bstart[blk_t])
                    last = ((t + 1) * 128 == bstart[blk_t + 1])
                    if first:
                        # full PSUM bank per in-flight accumulation group
                        acc_ps = apspool.tile([128, 512], F32, tag="aps")
                    nc.tensor.matmul(
                        out=acc_ps[:, 0:64],
                        lhsT=pslab[sg][:, t - sg * GRP, :],
                        rhs=contrib[sg][:, t - sg * GRP, 0:64],
                        start=first, stop=last, skip_group_check=True,
                    )
                    if last:
                        pending_copies.append((blk_t, acc_ps))
                pslab[sg] = None
                contrib[sg] = None
                for seg in range(nseg):
                    if g_tiles[seg] is not None and \
                            (seg + 1) * SEG <= (grp - 1) * GRP * 128:
                        g_tiles[seg] = None

            for blk_t, aps_h in pending_copies:
                blk_copy_rr += 1
                if blk_copy_rr % 2 == 0:
                    nc.vector.tensor_copy(
                        out=acc_sb[:, blk_t, 0:64], in_=aps_h[:, 0:64])
                else:
                    nc.scalar.activation(
                        out=acc_sb[:, blk_t, 0:64], in_=aps_h[:, 0:64],
                        func=mybir.ActivationFunctionType.Copy)
            for blk_t in pending_chains:
                _bn_chain(blk_t)
            for blk_t, _ in pending_copies:
                _bn_chain(blk_t)
            pending_copies = []

        # ---- BN phase ----
        with (
            tc.tile_pool(name="bn", bufs=1) as bnpool,
            tc.tile_pool(name="bnp", bufs=1, space="PSUM") as bnps,
        ):
            st0 = bnpool.tile([1, 64], F32, tag="st0")
            st1 = bnpool.tile([1, 64], F32, tag="st1")
            nc.vector.tensor_copy(out=st0[:, :], in_=gs_ps[64:65, 0:64])
            # Sum(x^2) = diag of the Gram matrix: mask with identity, reduce
            g_sb = bnpool.tile([64, 64], F32, tag="gsb")
            nc.scalar.activation(
                out=g_sb[:, :], in_=gs_ps[0:64, 0:64],
                func=mybir.ActivationFunctionType.Copy)
            id_sb = bnpool.tile([64, 64], F32, tag="idm")
            nc.vector.tensor_scalar(
                out=id_sb[:, :], in0=cio_sb[0:64, 0:64],
                scalar1=pidx_sb[0:64, :], scalar2=None,
                op0=mybir.AluOpType.is_equal)
            nc.vector.tensor_tensor(
                out=g_sb[:, :], in0=g_sb[:, :], in1=id_sb[:, :],
                op=mybir.AluOpType.mult)
            ones64 = bnpool.tile([64, 1], F32, tag="ones64")
            nc.vector.memset(ones64[:, :], 1.0)
            st_ps = bnps.tile([1, 512], F32, tag="stp")
            nc.tensor.matmul(
                out=st_ps[:, 0:64], lhsT=ones64[:, :], rhs=g_sb[:, :],
                start=True, stop=True, skip_group_check=True)
            nc.vector.tensor_copy(out=st1[:, :], in_=st_ps[:, 0:64])

            nc.sync.dma_start(out=cc_in[0:1, :], in_=st0[:, :])
            nc.sync.dma_start(out=cc_in[1:2, :], in_=st1[:, :])
            nc.gpsimd.collective_compute(
                "AllGather", mybir.AluOpType.bypass,
                ins=[cc_in[:, :]], outs=[cc_out[:, :]],
                replica_groups=[list(range(n_cores))],
            )
            gath = bnpool.tile([n_cores, 2, 64], F32, tag="gath")
            nc.sync.dma_start(out=gath[:, :, :], in_=cc_out[:, :])
            ones2 = bnpool.tile([n_cores, 1], F32, tag="ones2")
            nc.vector.memset(ones2[:, :], 1.0)
            gsum_ps = bnps.tile([1, 512], F32, tag="gsum")
            gs0 = bnpool.tile([1, 64], F32, tag="gs0")
            gs1 = bnpool.tile([1, 64], F32, tag="gs1")
            nc.tensor.matmul(
                out=gsum_ps[:, 0:64], lhsT=ones2[:, :], rhs=gath[:, 0, :],
                start=True, stop=True)
            nc.vector.tensor_copy(out=gs0[:, :], in_=gsum_ps[:, 0:64])
            nc.tensor.matmul(
                out=gsum_ps[:, 0:64], lhsT=ones2[:, :], rhs=gath[:, 1, :],
                start=True, stop=True)
            nc.vector.tensor_copy(out=gs1[:, :], in_=gsum_ps[:, 0:64])
            gam_t = bnpool.tile([1, 64], F32, tag="gam")
            bet_t = bnpool.tile([1, 64], F32, tag="bet")
            nc.sync.dma_start(out=gam_t[:, :], in_=gb_t[0:1, :])
            nc.sync.dma_start(out=bet_t[:, :], in_=gb_t[1:2, :])

            inv_n = 1.0 / float(n_out)
            mean_t = bnpool.tile([1, 64], F32, tag="mean")
            msq_t = bnpool.tile([1, 64], F32, tag="msq")
            var_t = bnpool.tile([1, 64], F32, tag="var")
            rs_t = bnpool.tile([1, 64], F32, tag="rs")
            a_t = bnpool.tile([1, 64], F32, tag="a")
            b_t = bnpool.tile([1, 64], F32, tag="b")
            nc.vector.tensor_scalar_mul(mean_t[:, :], gs0[:, :], inv_n)
            # msq = (gs0 * inv_n) * mean = mean^2 ; var = gs1*inv_n - msq
            nc.vector.scalar_tensor_tensor(
                out=msq_t[:, :], in0=gs0[:, :], scalar=inv_n,
                in1=mean_t[:, :], op0=mybir.AluOpType.mult,
                op1=mybir.AluOpType.mult)
            nc.vector.scalar_tensor_tensor(
                out=var_t[:, :], in0=gs1[:, :], scalar=inv_n,
                in1=msq_t[:, :], op0=mybir.AluOpType.mult,
                op1=mybir.AluOpType.subtract)
            sd_t = bnpool.tile([1, 64], F32, tag="sd")
            eps_t = bnpool.tile([1, 1], F32, tag="eps")
            nc.vector.memset(eps_t[:, :], BN_EPS)
            nc.scalar.activation(
                out=sd_t[:, :], in_=var_t[:, :],
                func=mybir.ActivationFunctionType.Sqrt, bias=eps_t[:, :])
            nc.vector.reciprocal(out=rs_t[:, :], in_=sd_t[:, :])
            nc.vector.tensor_tensor(
                out=a_t[:, :], in0=gam_t[:, :], in1=rs_t[:, :],
                op=mybir.AluOpType.mult)
            nc.vector.tensor_tensor(
                out=b_t[:, :], in0=mean_t[:, :], in1=a_t[:, :],
                op=mybir.AluOpType.mult)
            nc.vector.tensor_tensor(
                out=b_t[:, :], in0=bet_t[:, :], in1=b_t[:, :],
                op=mybir.AluOpType.subtract)
            ones_row = bnpool.tile([1, 128], F32, tag="ones_row")
            nc.vector.memset(ones_row[:, :], 1.0)
            a_full = bnpool.tile([128, 64], BF16, tag="afull")
            b_full = bnpool.tile([128, 64], BF16, tag="bfull")
            ab_ps = bnps.tile([128, 64], F32, tag="abps")
            nc.tensor.matmul(
                out=ab_ps[:, :], lhsT=ones_row[:, :], rhs=a_t[:, :],
                start=True, stop=True)
            nc.vector.tensor_copy(out=a_full[:, :], in_=ab_ps[:, :])
            nc.tensor.matmul(
                out=ab_ps[:, :], lhsT=ones_row[:, :], rhs=b_t[:, :],
                start=True, stop=True)
            nc.vector.tensor_copy(out=b_full[:, :], in_=ab_ps[:, :])
            # materialize per-channel a/b replicated along the block dim so
            # the chunked TT ops keep packed last-dims (DVE 2x mode)
            NCH = 24
            cw = (NBLK + NCH - 1) // NCH
            a_rep = bnpool.tile([128, cw, 64], BF16, tag="arep")
            b_rep = bnpool.tile([128, cw, 64], BF16, tag="brep")
            nc.vector.tensor_copy(
                out=a_rep[:, :, :],
                in_=a_full[:, :].unsqueeze(1).broadcast_to([128, cw, 64]))
            nc.vector.tensor_copy(
                out=b_rep[:, :, :],
                in_=b_full[:, :].unsqueeze(1).broadcast_to([128, cw, 64]))
            for ci in range(NCH):
                u0, u1 = ci * cw, min((ci + 1) * cw, NBLK)
                if u0 >= u1:
                    continue
                eng_m = nc.gpsimd if ci % 4 == 0 else nc.vector
                eng_m.tensor_tensor(
                    out=acc_sb[:, u0:u1, 0:64], in0=acc_sb[:, u0:u1, 0:64],
                    in1=a_rep[:, : u1 - u0, :],
                    op=mybir.AluOpType.mult)
                nc.vector.tensor_tensor(
                    out=acc_sb[:, u0:u1, 0:64], in0=acc_sb[:, u0:u1, 0:64],
                    in1=b_rep[:, : u1 - u0, :],
                    op=mybir.AluOpType.add)
                nc.scalar.activation(
                    out=acc_sb[:, u0:u1, 0:64],
                    in_=acc_sb[:, u0:u1, 0:64],
                    func=mybir.ActivationFunctionType.Relu)
                nc.sync.dma_start(out=y[:, u0 * 65:u1 * 65],
                                  in_=acc_sb[:, u0:u1, :])

    nc.compile()
    return nc


def _prepare(feats, W, gamma, beta, in_map, out_map, n_out, n_cores,
             dup_safe=False, expand=1):
    """Host prep. Returns (nc, in_maps, plan)."""
    n_out = int(n_out)
    K, Cin, Cout = W.shape
    assert Cin == 64 and Cout == 64
    in_map = np.asarray(in_map, dtype=np.int64)
    out_map = np.asarray(out_map, dtype=np.int64)
    feats = np.asarray(feats, dtype=np.float32)
    W = np.asarray(W, dtype=np.float32)
    J = (K + 1) // 2

    plan, per_core = _route(in_map, out_map, n_out, n_cores)
    S, ntiles = plan["S"], plan["ntiles"]

    feats_bf = feats.astype(ml_dtypes.bfloat16)

    wt = np.zeros((128, J * 64), dtype=ml_dtypes.bfloat16)
    for j in range(J):
        wt[0:64, j * 64:(j + 1) * 64] = W[2 * j].astype(ml_dtypes.bfloat16)
        if 2 * j + 1 < K:
            wt[64:128, j * 64:(j + 1) * 64] = (
                W[2 * j + 1].astype(ml_dtypes.bfloat16))

    cio = np.tile(np.arange(128, dtype=np.float32), (128, 1)).astype(
        ml_dtypes.bfloat16)
    gb = np.stack([np.asarray(gamma, np.float32),
                   np.asarray(beta, np.float32)])

    in_maps = []
    for c in range(n_cores):
        p = per_core[c]
        ftab = np.zeros((128, S), dtype=ml_dtypes.bfloat16)
        real = p["feat_row"] >= 0
        rows = p["feat_row"][real]
        halves = p["half"][real]
        cols = np.nonzero(real)[0]
        lo = halves == 0
        ftab[0:64, cols[lo]] = feats_bf[rows[lo]].T
        ftab[64:128, cols[~lo]] = feats_bf[rows[~lo]].T
        ridx = np.zeros((128, ntiles), dtype=np.float32)
        ridx[:, :] = p["ridx"].reshape(ntiles, 128).T
        pidx = np.arange(128, dtype=np.float32).reshape(128, 1)
        in_maps.append(dict(ftab=ftab, wt=wt, ridx=ridx, cio=cio, gb=gb,
                            pidx=pidx))

    nc = _build(plan, n_out, n_cores)
    return nc, in_maps, plan


def _unpack_y(res, plan, n_cores):
    NBLK = plan["NBLK"]
    row_order = plan["row_order"]       # [n_cores, NBLK, 128]
    n_out = row_order.max() + 1
    y_full = np.zeros((int(n_out), 64), np.float32)
    for c in range(n_cores):
        yc = np.asarray(res.results[c]["y"]).reshape(128, NBLK, 65)[:, :, :64]
        ro = row_order[c]               # [NBLK, 128]
        valid = ro >= 0
        y_full[ro[valid]] = yc.transpose(1, 0, 2)[valid].astype(np.float32)
    return y_full


def kernel(feats, W, gamma, beta, in_map, out_map, n_out):
    from concourse.bass_utils import run_bass_kernel_spmd

    n_cores = 8
    nc, in_maps, plan = _prepare(
        feats, W, gamma, beta, in_map, out_map, int(n_out), n_cores)
    res = run_bass_kernel_spmd(nc, in_maps, list(range(n_cores)))
    return _unpack_y(res, plan, n_cores)


# revision 50
# speedup vs baseline: 1.0187x; 1.0187x over previous
"""Trainium2 Bass kernel for nn_BasicDeconvolutionBlock.

Reference computation:
    gathered = feats[in_map]                         # [K, M, Cin]
    contrib  = einsum('kmc,kcd->kmd', gathered, W)   # [K, M, Cout]
    out      = zeros([n_out, Cout]).at[out_map].add(contrib)
    y        = relu(batchnorm(out))                  # batch stats over n_out rows

Strategy v2 (8 NeuronCores, SPMD, matmul-scatter, balanced routing):
  - Output rows are assigned to 1632 bins (204 per core, <=128 rows each)
    by a host-side balancer (shave + reinsert + swap) so that EVERY
    (bin, k-pair) group has <= 64 pairs on every core.  Group caps are
    then uniformly 64 (or 32), eliminating the 32-align/96-bump padding
    of v1: S/core = ~183k slots vs 227k.
  - k-PAIR PACKING: offsets (2j, 2j+1) share one GEMM with stacked
    weights [W_2j; W_2j+1] ([128, 64]); a pair's feats occupy channel
    half k%2, zeros in the other half.
  - Groups ordered caps-64-first within each block so every group is a
    single legal matmul run (PSUM partition bases 0/64 for 64-wide,
    0/32/64/96 for 32-wide).  Blocks pad to 128-slot tiles.
  - Blocks hold <=128 output rows -> scatter one-hot P is [128 x 128]
    and each tile needs ONE scatter matmul (v1: [128 x 256], two).
  - P tiles via tensor_scalar(is_equal) vs an iota constant, split
    ~70/30 across DVE and Pool (gpsimd).  Pad slots (ridx=-1) give zero
    columns.
  - Host pre-gathers feats into channel-major ftab[128, S] bf16,
    streamed sequentially in large segments.
  - Software pipelining: batch g's scatters issue after batch g+1's
    mains (hides the contrib PSUM->SBUF copy); acc copies lag one
    iteration and BN chains two, so no engine waits on another's
    just-issued work (engine queues are in-order).
  - BN: every acc block carries a constant ones-column (65 wide); one
    chained self-Gram matmul per block yields Sum(x^2) on the diagonal
    and Sum(x) in row 64.  [2,64] partials AllGathered then reduced
    on-chip; normalize (mult/add on DVE+Pool, ReLU on Act) chunked and
    overlapped with the bf16 y writeback.
"""

import numpy as np

import sys

sys.path.insert(0, "/opt/trn_rl_repo")

import ml_dtypes  # noqa: E402

from concourse import bacc, mybir  # noqa: E402
import concourse.tile as tile  # noqa: E402

BN_EPS = 1e-5
F32 = mybir.dt.float32
BF16 = mybir.dt.bfloat16

NBINS_PER_CORE = 204
CAP = 64
ROWCAP = 128


def _roundup(x, m):
    return (x + m - 1) // m * m


def _balance_rows(deg, n_out, nbins, rng):
    """Assign rows to bins s.t. every (bin, j) count <= CAP and rows <= ROWCAP.

    Returns assign [n_out] -> bin id.
    """
    J = deg.shape[1]
    assign = np.empty(n_out, np.int32)
    perm = rng.permutation(n_out)
    assign[perm] = np.arange(n_out) % nbins
    binj = np.zeros((nbins, J), np.int64)
    for j in range(J):
        np.add.at(binj[:, j], assign, deg[:, j])
    binrows = np.bincount(assign, minlength=nbins).astype(np.int64)

    TRIM = CAP - 6
    pool = []
    rows_by_bin = [list(np.nonzero(assign == b)[0]) for b in range(nbins)]
    for b in range(nbins):
        guard = 0
        while (binj[b] > TRIM).any() and guard < 400:
            guard += 1
            jbad = int(np.argmax(binj[b]))
            cand = [r for r in rows_by_bin[b] if deg[r, jbad] > 0]
            if not cand:
                break
            r = max(cand, key=lambda r: deg[r, jbad])
            rows_by_bin[b].remove(r)
            pool.append(r)
            binj[b] -= deg[r]
            binrows[b] -= 1
            assign[r] = -1

    pool.sort(key=lambda r: -deg[r].max())
    hard = []
    for r in pool:
        v = deg[r]
        js = np.nonzero(v)[0]
        ok = binrows < ROWCAP
        for j in js:
            ok &= binj[:, j] + v[j] <= CAP - 1
        cand = np.nonzero(ok)[0]
        if len(cand) == 0:
            hard.append(r)
            continue
        t = int(cand[np.argmin(binj[cand].max(1))])
        assign[r] = t
        binj[t] += v
        binrows[t] += 1
    for r in hard:
        v = deg[r]
        js = np.nonzero(v)[0]
        ok = binrows < ROWCAP
        for j in js:
            ok &= binj[:, j] + v[j] <= CAP
        cand = np.nonzero(ok)[0]
        if len(cand) == 0:
            score = np.maximum(binj[:, js] + v[js][None, :] - CAP, 0).sum(1) + \
                np.where(binrows >= ROWCAP, 10000, 0)
            t = int(np.argmin(score))
        else:
            t = int(cand[np.argmin(binj[cand].max(1))])
        assign[r] = t
        binj[t] += v
        binrows[t] += 1

    # swap repair for any residual over-cap groups
    for _ in range(6):
        over = np.argwhere(binj > CAP)
        if len(over) == 0:
            break
        for b, j in over:
            guard = 0
            while binj[b, j] > CAP and guard < 20:
                guard += 1
                rows_b = np.nonzero((assign == b) & (deg[:, j] > 0))[0]
                r = rows_b[np.argmax(deg[rows_b, j])]
                v = deg[r]
                js = np.nonzero(v)[0]
                ok = binrows < ROWCAP
                for jj in js:
                    ok &= binj[:, jj] + v[jj] <= CAP
                ok[b] = False
                cand = np.nonzero(ok)[0]
                if len(cand):
                    t = int(cand[np.argmin(binj[cand].max(1))])
                    assign[r] = t
                    binj[t] += v
                    binrows[t] += 1
                    binj[b] -= v
                    binrows[b] -= 1
                    continue
                done = False
                for t in np.argsort(binj[:, j])[:60]:
                    if t == b:
                        continue
                    rows_t = np.nonzero(assign == t)[0]
                    for r2 in rows_t[np.argsort(deg[rows_t].max(1))[:40]]:
                        v2 = deg[r2]
                        nb = binj[b] - v + v2
                        nt = binj[t] - v2 + v
                        if (nb <= CAP).all() and (nt <= CAP).all():
                            assign[r] = t
                            assign[r2] = b
                            binj[b] = nb
                            binj[t] = nt
                            done = True
                            break
                    if done:
                        break
                if not done:
                    break
    return assign, binj


def _route(in_map, out_map, n_out, n_cores):
    """Host routing v2. Returns (plan, per-core slot tables, row_order)."""
    K, M = in_map.shape
    J = (K + 1) // 2
    nbins = NBINS_PER_CORE * n_cores
    NBLK = NBINS_PER_CORE

    k_idx = np.repeat(np.arange(K, dtype=np.int64), M)
    in_flat = in_map.ravel().astype(np.int64)
    out_flat = out_map.ravel().astype(np.int64)
    j_idx = k_idx // 2
    half = (k_idx % 2).astype(np.int64)

    deg = np.zeros((n_out, J), np.int64)
    np.add.at(deg, (out_flat, j_idx), 1)

    rng = np.random.default_rng(0)
    assign, binj = _balance_rows(deg, n_out, nbins, rng)

    # bins -> (core, position): sort by total load, deal round-robin
    ordv = np.argsort(-binj.sum(1), kind="stable")
    core_of_bin = np.empty(nbins, np.int32)
    pos_of_bin = np.empty(nbins, np.int32)
    for i, b in enumerate(ordv):
        core_of_bin[b] = i % n_cores
        pos_of_bin[b] = i // n_cores

    # caps[pos, j] = roundup32(max over cores of binj)
    posmax = np.zeros((NBLK, J), np.int64)
    np.maximum.at(posmax, pos_of_bin, binj)
    caps = ((posmax + 31) // 32) * 32  # 0 / 32 / 64
    # order groups within a block: 64-caps first, then 32s (single legal runs)
    jorder = [np.argsort(-caps[p], kind="stable") for p in range(NBLK)]

    # block slot layout
    goff = np.zeros((NBLK, J), np.int64)     # slot offset of group (pos, j)
    btot = np.zeros(NBLK, np.int64)
    for p in range(NBLK):
        s = 0
        for j in jorder[p]:
            goff[p, j] = s
            s += caps[p, j]
        btot[p] = _roundup(s, 128)
    bstart = np.concatenate([[0], np.cumsum(btot)])
    S = int(bstart[-1])
    ntiles = S // 128
    goff += bstart[:-1][:, None]

    # tiles: per tile (block, [(j, c0, c1) runs]); caps 64/32 at 32-mult
    # offsets never need splitting (bases 0/32/64/96 all legal for w<=32,
    # 0/64 for w<=64)
    tile_block = np.searchsorted(bstart, np.arange(ntiles) * 128, side="right") - 1
    runs_per_tile = [[] for _ in range(ntiles)]
    for p in range(NBLK):
        for j in range(J):
            c = int(caps[p, j])
            if c == 0:
                continue
            g0 = int(goff[p, j])
            # split at tile boundaries, then at legal PSUM quadrant bases
            # (0: any width, 32: <=32, 64: <=64, 96: <=32)
            pos = g0
            while pos < g0 + c:
                t = pos // 128
                c0 = pos - t * 128
                c1 = min(g0 + c - t * 128, 128)
                while c0 < c1:
                    if c0 == 0:
                        e = c1
                    elif c0 == 32:
                        e = min(c1, 64)
                    elif c0 == 64:
                        e = min(c1, 128)
                    elif c0 == 96:
                        e = c1
                    else:
                        raise AssertionError((p, j, c0, c1))
                    runs_per_tile[t].append((int(j), c0, e))
                    c0 = e
                pos = t * 128 + c1
    tiles = [(int(tile_block[t]), runs_per_tile[t]) for t in range(ntiles)]

    # per-core slot tables + row ordering
    # row index within bin: sorted ascending global row id
    order_rows = np.argsort(assign, kind="stable")
    rows_sorted = order_rows[assign[order_rows] >= 0]
    # rows grouped by bin
    bin_of = assign[rows_sorted]
    starts = np.searchsorted(bin_of, np.arange(nbins))
    ends = np.searchsorted(bin_of, np.arange(nbins) + 1)
    ridx_of_row = np.full(n_out, -1, np.int64)
    for b in range(nbins):
        rs = rows_sorted[starts[b]:ends[b]]
        ridx_of_row[rs] = np.arange(len(rs))

    # row_order[core, pos] = global rows (for host unpack)
    row_order = np.full((n_cores, NBLK, ROWCAP), -1, np.int64)
    for b in range(nbins):
        rs = rows_sorted[starts[b]:ends[b]]
        row_order[core_of_bin[b], pos_of_bin[b], :len(rs)] = rs

    bin_of_pair = assign[out_flat]
    core_of_pair = core_of_bin[bin_of_pair]
    pos_of_pair = pos_of_bin[bin_of_pair]
    rix_of_pair = ridx_of_row[out_flat]
    assert rix_of_pair.max() < ROWCAP and rix_of_pair.min() >= 0

    per_core = []
    for c in range(n_cores):
        sel = np.nonzero(core_of_pair == c)[0]
        o = np.lexsort((in_flat[sel], j_idx[sel], pos_of_pair[sel]))
        sel = sel[o]
        cb, cj = pos_of_pair[sel], j_idx[sel]
        key = cb * J + cj
        n = len(key)
        first = np.ones(n, dtype=bool)
        first[1:] = key[1:] != key[:-1]
        grp_start = np.maximum.accumulate(np.where(first, np.arange(n), 0))
        pos = np.arange(n) - grp_start
        slots = goff[cb, cj] + pos
        assert (pos < caps[cb, cj]).all()

        feat_row = np.full(S, -1, np.int64)
        s_half = np.zeros(S, np.int64)
        ridx = np.full(S, -1.0, np.float32)
        feat_row[slots] = in_flat[sel]
        s_half[slots] = half[sel]
        ridx[slots] = rix_of_pair[sel].astype(np.float32)
        per_core.append(dict(feat_row=feat_row, half=s_half, ridx=ridx))

    plan = dict(
        S=S, ntiles=ntiles, NBLK=NBLK, J=J,
        tiles=tiles, bstart=bstart.tolist(),
        row_order=row_order,
        # test.py compat
        R=1, total_slots=S, segments=[None] * ((S + 4095) // 4096),
        rows_per_core=n_out // n_cores, acc_rows=NBLK * 128,
    )
    return plan, per_core


def _build(plan, n_out, n_cores):
    nc = bacc.Bacc("TRN2", target_bir_lowering=False, debug=False)

    S, ntiles, NBLK, J = plan["S"], plan["ntiles"], plan["NBLK"], plan["J"]
    tiles = plan["tiles"]
    bstart = plan["bstart"]
    SEG = 4096   # slots per ftab DMA
    GRP = 16     # tiles per contrib PSUM batch ([128, 1024] f32 = 2 banks)

    ftab = nc.dram_tensor("ftab", [128, S], BF16, kind="ExternalInput")
    wt = nc.dram_tensor("wt", [128, J * 64], BF16, kind="ExternalInput")
    ridx = nc.dram_tensor("ridx", [128, ntiles], F32, kind="ExternalInput")
    cio = nc.dram_tensor("cio", [128, 128], BF16, kind="ExternalInput")
    pidx = nc.dram_tensor("pidx", [128, 1], F32, kind="ExternalInput")
    gb_t = nc.dram_tensor("gb", [2, 64], F32, kind="ExternalInput")
    cc_in = nc.dram_tensor("cc_in", [2, 64], F32)
    cc_out = nc.dram_tensor("cc_out", [2 * n_cores, 64], F32, addr_space="Shared")
    y = nc.dram_tensor("y", [128, NBLK * 65], BF16, kind="ExternalOutput")

    nseg = (S + SEG - 1) // SEG
    ngrp = (ntiles + GRP - 1) // GRP

    with tile.TileContext(nc) as tc:
      with tc.tile_pool(name="acc", bufs=1) as apool, \
           tc.tile_pool(name="bnch", bufs=1, space="PSUM") as bnchpool:
        with (
            tc.tile_pool(name="const", bufs=1) as cpool,
            tc.tile_pool(name="g", bufs=8) as gpool,
            tc.tile_pool(name="cp", bufs=5) as csbpool,
            tc.tile_pool(name="pg", bufs=5) as ppool,
            tc.tile_pool(name="cps", bufs=2, space="PSUM") as cpspool,
            tc.tile_pool(name="aps", bufs=3, space="PSUM") as apspool,
        ):
            w_sb = cpool.tile([128, J * 64], BF16, tag="w")
            nc.sync.dma_start(out=w_sb[:, :], in_=wt[:, :])
            g_tiles = [None] * nseg
            g0 = gpool.tile([128, SEG], BF16, tag="g")
            nc.sync.dma_start(out=g0[:, :SEG], in_=ftab[:, 0:SEG])
            g_tiles[0] = (g0, 0)
            cio_sb = apool.tile([128, 128], BF16, tag="cio")
            nc.sync.dma_start(out=cio_sb[:, :], in_=cio[:, :])
            ridx_sb = cpool.tile([128, ntiles], F32, tag="ridx")
            for rc in range(4):
                r0 = rc * ((ntiles + 3) // 4)
                r1 = min(r0 + (ntiles + 3) // 4, ntiles)
                if r0 < r1:
                    nc.sync.dma_start(out=ridx_sb[:, r0:r1],
                                      in_=ridx[:, r0:r1])
            pidx_sb = apool.tile([128, 1], F32, tag="pidx")
            nc.sync.dma_start(out=pidx_sb[:, :], in_=pidx[:, :])
            warm = cpool.tile([1, 1], F32, tag="warm")
            nc.vector.memset(warm[:, :], 1.0)
            nc.scalar.activation(
                out=warm[:, :], in_=warm[:, :],
                func=mybir.ActivationFunctionType.Sqrt)
            acc_sb = apool.tile([128, NBLK, 65], BF16, tag="acc")
            # col 64 of every block is a constant 1.0: the per-block
            # self-Gram matmul then yields Sum(x^2) on its diagonal AND
            # Sum(x) in row 64, one chained matmul per block
            nc.vector.memset(acc_sb[:, :, 64:65], 1.0)
            gs_ps = bnchpool.tile([128, 512], F32, tag="gs")

            contrib = [None] * ngrp
            pslab = [None] * ngrp
            acc_ps = None
            pending_copies = []
            pending_chains = []
            rr = 0
            blk_copy_rr = 0

            def _bn_chain(blk):
                # chained BN stats: [gram | Sum(x)] in one matmul
                nc.tensor.matmul(
                    out=gs_ps[0:65, 0:65],
                    lhsT=acc_sb[:, blk, :],
                    rhs=acc_sb[:, blk, :],
                    start=(blk == 0), stop=(blk == NBLK - 1),
                    skip_group_check=True)

            # software-pipelined: batch g's scatters are issued AFTER batch
            # g+1's mains, so the contrib PSUM->SBUF copy latency hides
            # behind the next batch's PE work instead of stalling scatters
            for grp in range(ngrp + 1):
                if grp < ngrp:
                    t0 = grp * GRP
                    t1 = min(t0 + GRP, ntiles)
                    cps = cpspool.tile([128, GRP * 64], F32, tag="cps")
                    P = ppool.tile([128, GRP, 128], BF16, tag="P")
                    csb_t = csbpool.tile([128, GRP, 64], BF16, tag="csb")
                    contrib[grp] = csb_t
                    half = t0 + (t1 - t0 + 1) // 2
                    for t in range(t0, t1):
                        seg = t * 128 // SEG
                        if g_tiles[seg] is None:
                            s0 = seg * SEG
                            s1 = min(s0 + SEG, S)
                            g = gpool.tile([128, SEG], BF16, tag="g")
                            nc.sync.dma_start(out=g[:, : s1 - s0],
                                              in_=ftab[:, s0:s1])
                            g_tiles[seg] = (g, s0)
                        g, s0 = g_tiles[seg]
                        toff = t * 128 - s0
                        blk_t, runs = tiles[t]
                        for (j, c0, c1) in runs:
                            nc.tensor.matmul(
                                out=cps[c0:c1, (t - t0) * 64:(t - t0 + 1) * 64],
                                lhsT=g[:, toff + c0: toff + c1],
                                rhs=w_sb[:, j * 64:(j + 1) * 64],
                                start=True, stop=True, skip_group_check=True,
                            )
                        # P tile: 1[cio[r] == ridx[s]]; pads -> zero cols
                        eng = nc.vector if rr % 25 < 17 else nc.gpsimd
                        rr += 1
                        eng.tensor_scalar(
                            out=P[:, t - t0, :], in0=cio_sb[:, :],
                            scalar1=ridx_sb[:, t:t + 1], scalar2=None,
                            op0=mybir.AluOpType.is_equal,
                        )
                    nc.scalar.activation(
                        out=csb_t[:, : t1 - t0, :],
                        in_=cps[:, : (t1 - t0) * 64],
                        func=mybir.ActivationFunctionType.Copy,
                    )
                    pslab[grp] = P

                # acc copies for blocks completed LAST iteration (scatter
                # deps a full batch old -> DVE/Act don't stall on PE), then
                # BN chains for blocks copied the iteration BEFORE (copy
                # deps a full batch old -> PE doesn't stall on DVE/Act)
                for blk_t, aps_h in pending_copies:
                    blk_copy_rr += 1
                    if blk_copy_rr % 5 < 2:
                        nc.vector.tensor_copy(
                            out=acc_sb[:, blk_t, 0:64], in_=aps_h[:, 0:64])
                    else:
                        nc.scalar.activation(
                            out=acc_sb[:, blk_t, 0:64], in_=aps_h[:, 0:64],
                            func=mybir.ActivationFunctionType.Copy)
                for blk_t in pending_chains:
                    _bn_chain(blk_t)
                pending_chains = [b for b, _ in pending_copies]
                pending_copies = []

                if grp == 0:
                    continue
                # scatter the PREVIOUS batch's tiles (one matmul each, B=128)
                sg = grp - 1
                for t in range(sg * GRP, min(sg * GRP + GRP, ntiles)):
                    blk_t, _ = tiles[t]
                    first = (t * 128 == bstart[blk_t])
                    last = ((t + 1) * 128 == bstart[blk_t + 1])
                    if first:
                        # full PSUM bank per in-flight accumulation group
                        acc_ps = apspool.tile([128, 512], F32, tag="aps")
                    nc.tensor.matmul(
                        out=acc_ps[:, 0:64],
                        lhsT=pslab[sg][:, t - sg * GRP, :],
                        rhs=contrib[sg][:, t - sg * GRP, 0:64],
                        start=first, stop=last, skip_group_check=True,
                    )
                    if last:
                        pending_copies.append((blk_t, acc_ps))
                pslab[sg] = None
                contrib[sg] = None
                for seg in range(nseg):
                    if g_tiles[seg] is not None and \
                            (seg + 1) * SEG <= (grp - 1) * GRP * 128:
                        g_tiles[seg] = None

            for blk_t, aps_h in pending_copies:
                blk_copy_rr += 1
                if blk_copy_rr % 2 == 0:
                    nc.vector.tensor_copy(
                        out=acc_sb[:, blk_t, 0:64], in_=aps_h[:, 0:64])
                else:
                    nc.scalar.activation(
                        out=acc_sb[:, blk_t, 0:64], in_=aps_h[:, 0:64],
                        func=mybir.ActivationFunctionType.Copy)
            for blk_t in pending_chains:
                _bn_chain(blk_t)
            for blk_t, _ in pending_copies:
                _bn_chain(blk_t)
            pending_copies = []

        # ---- BN phase ----
        with (
            tc.tile_pool(name="bn", bufs=1) as bnpool,
            tc.tile_pool(name="bnp", bufs=1, space="PSUM") as bnps,
        ):
            st0 = bnpool.tile([1, 64], F32, tag="st0")
            st1 = bnpool.tile([1, 64], F32, tag="st1")
            nc.vector.tensor_copy(out=st0[:, :], in_=gs_ps[64:65, 0:64])
            # Sum(x^2) = diag of the Gram matrix: mask with identity, reduce
            g_sb = bnpool.tile([64, 64], F32, tag="gsb")
            nc.scalar.activation(
                out=g_sb[:, :], in_=gs_ps[0:64, 0:64],
                func=mybir.ActivationFunctionType.Copy)
            id_sb = bnpool.tile([64, 64], F32, tag="idm")
            nc.vector.tensor_scalar(
                out=id_sb[:, :], in0=cio_sb[0:64, 0:64],
                scalar1=pidx_sb[0:64, :], scalar2=None,
                op0=mybir.AluOpType.is_equal)
            nc.vector.tensor_tensor(
                out=g_sb[:, :], in0=g_sb[:, :], in1=id_sb[:, :],
                op=mybir.AluOpType.mult)
            ones64 = bnpool.tile([64, 1], F32, tag="ones64")
            nc.vector.memset(ones64[:, :], 1.0)
            st_ps = bnps.tile([1, 512], F32, tag="stp")
            nc.tensor.matmul(
                out=st_ps[:, 0:64], lhsT=ones64[:, :], rhs=g_sb[:, :],
                start=True, stop=True, skip_group_check=True)
            nc.vector.tensor_copy(out=st1[:, :], in_=st_ps[:, 0:64])

            nc.sync.dma_start(out=cc_in[0:1, :], in_=st0[:, :])
            nc.sync.dma_start(out=cc_in[1:2, :], in_=st1[:, :])
            nc.gpsimd.collective_compute(
                "AllGather", mybir.AluOpType.bypass,
                ins=[cc_in[:, :]], outs=[cc_out[:, :]],
                replica_groups=[list(range(n_cores))],
            )
            gath = bnpool.tile([n_cores, 2, 64], F32, tag="gath")
            nc.sync.dma_start(out=gath[:, :, :], in_=cc_out[:, :])
            ones2 = bnpool.tile([n_cores, 1], F32, tag="ones2")
            nc.vector.memset(ones2[:, :], 1.0)
            gsum_ps = bnps.tile([1, 512], F32, tag="gsum")
            gs0 = bnpool.tile([1, 64], F32, tag="gs0")
            gs1 = bnpool.tile([1, 64], F32, tag="gs1")
            nc.tensor.matmul(
                out=gsum_ps[:, 0:64], lhsT=ones2[:, :], rhs=gath[:, 0, :],
                start=True, stop=True)
            nc.vector.tensor_copy(out=gs0[:, :], in_=gsum_ps[:, 0:64])
            nc.tensor.matmul(
                out=gsum_ps[:, 0:64], lhsT=ones2[:, :], rhs=gath[:, 1, :],
                start=True, stop=True)
            nc.vector.tensor_copy(out=gs1[:, :], in_=gsum_ps[:, 0:64])
            gam_t = bnpool.tile([1, 64], F32, tag="gam")
            bet_t = bnpool.tile([1, 64], F32, tag="bet")
            nc.sync.dma_start(out=gam_t[:, :], in_=gb_t[0:1, :])
            nc.sync.dma_start(out=bet_t[:, :], in_=gb_t[1:2, :])

            inv_n = 1.0 / float(n_out)
            mean_t = bnpool.tile([1, 64], F32, tag="mean")
            msq_t = bnpool.tile([1, 64], F32, tag="msq")
            var_t = bnpool.tile([1, 64], F32, tag="var")
            rs_t = bnpool.tile([1, 64], F32, tag="rs")
            a_t = bnpool.tile([1, 64], F32, tag="a")
            b_t = bnpool.tile([1, 64], F32, tag="b")
            nc.vector.tensor_scalar_mul(mean_t[:, :], gs0[:, :], inv_n)
            # msq = (gs0 * inv_n) * mean = mean^2 ; var = gs1*inv_n - msq
            nc.vector.scalar_tensor_tensor(
                out=msq_t[:, :], in0=gs0[:, :], scalar=inv_n,
                in1=mean_t[:, :], op0=mybir.AluOpType.mult,
                op1=mybir.AluOpType.mult)
            nc.vector.scalar_tensor_tensor(
                out=var_t[:, :], in0=gs1[:, :], scalar=inv_n,
                in1=msq_t[:, :], op0=mybir.AluOpType.mult,
                op1=mybir.AluOpType.subtract)
            sd_t = bnpool.tile([1, 64], F32, tag="sd")
            eps_t = bnpool.tile([1, 1], F32, tag="eps")
            nc.vector.memset(eps_t[:, :], BN_EPS)
            nc.scalar.activation(
                out=sd_t[:, :], in_=var_t[:, :],
                func=mybir.ActivationFunctionType.Sqrt, bias=eps_t[:, :])
            nc.vector.reciprocal(out=rs_t[:, :], in_=sd_t[:, :])
            nc.vector.tensor_tensor(
                out=a_t[:, :], in0=gam_t[:, :], in1=rs_t[:, :],
                op=mybir.AluOpType.mult)
            nc.vector.tensor_tensor(
                out=b_t[:, :], in0=mean_t[:, :], in1=a_t[:, :],
                op=mybir.AluOpType.mult)
            nc.vector.tensor_tensor(
                out=b_t[:, :], in0=bet_t[:, :], in1=b_t[:, :],
                op=mybir.AluOpType.subtract)
            ones_row = bnpool.tile([1, 128], F32, tag="ones_row")
            nc.vector.memset(ones_row[:, :], 1.0)
            a_full = bnpool.tile([128, 64], BF16, tag="afull")
            b_full = bnpool.tile([128, 64], BF16, tag="bfull")
            ab_ps = bnps.tile([128, 64], F32, tag="abps")
            nc.tensor.matmul(
                out=ab_ps[:, :], lhsT=ones_row[:, :], rhs=a_t[:, :],
                start=True, stop=True)
            nc.vector.tensor_copy(out=a_full[:, :], in_=ab_ps[:, :])
            nc.tensor.matmul(
                out=ab_ps[:, :], lhsT=ones_row[:, :], rhs=b_t[:, :],
                start=True, stop=True)
            nc.vector.tensor_copy(out=b_full[:, :], in_=ab_ps[:, :])
            # materialize per-channel a/b replicated along the block dim so
            # the chunked TT ops keep packed last-dims (DVE 2x mode)
            NCH = 24
            cw = (NBLK + NCH - 1) // NCH
            a_rep = bnpool.tile([128, cw, 64], BF16, tag="arep")
            b_rep = bnpool.tile([128, cw, 64], BF16, tag="brep")
            nc.vector.tensor_copy(
                out=a_rep[:, :, :],
                in_=a_full[:, :].unsqueeze(1).broadcast_to([128, cw, 64]))
            nc.vector.tensor_copy(
                out=b_rep[:, :, :],
                in_=b_full[:, :].unsqueeze(1).broadcast_to([128, cw, 64]))
            for ci in range(NCH):
                u0, u1 = ci * cw, min((ci + 1) * cw, NBLK)
                if u0 >= u1:
                    continue
                eng_m = nc.gpsimd if ci % 4 == 0 else nc.vector
                eng_m.tensor_tensor(
                    out=acc_sb[:, u0:u1, 0:64], in0=acc_sb[:, u0:u1, 0:64],
                    in1=a_rep[:, : u1 - u0, :],
                    op=mybir.AluOpType.mult)
                nc.vector.tensor_tensor(
                    out=acc_sb[:, u0:u1, 0:64], in0=acc_sb[:, u0:u1, 0:64],
                    in1=b_rep[:, : u1 - u0, :],
                    op=mybir.AluOpType.add)
                nc.scalar.activation(
                    out=acc_sb[:, u0:u1, 0:64],
                    in_=acc_sb[:, u0:u1, 0:64],
                    func=mybir.ActivationFunctionType.Relu)
                nc.sync.dma_start(out=y[:, u0 * 65:u1 * 65],
                                  in_=acc_sb[:, u0:u1, :])

    nc.compile()
    return nc


def _prepare(feats, W, gamma, beta, in_map, out_map, n_out, n_cores,
             dup_safe=False, expand=1):
    """Host prep. Returns (nc, in_maps, plan)."""
    n_out = int(n_out)
    K, Cin, Cout = W.shape
    assert Cin == 64 and Cout == 64
    in_map = np.asarray(in_map, dtype=np.int64)
    out_map = np.asarray(out_map, dtype=np.int64)
    feats = np.asarray(feats, dtype=np.float32)
    W = np.asarray(W, dtype=np.float32)
    J = (K + 1) // 2

    plan, per_core = _route(in_map, out_map, n_out, n_cores)
    S, ntiles = plan["S"], plan["ntiles"]

    feats_bf = feats.astype(ml_dtypes.bfloat16)

    wt = np.zeros((128, J * 64), dtype=ml_dtypes.bfloat16)
    for j in range(J):
        wt[0:64, j * 64:(j + 1) * 64] = W[2 * j].astype(ml_dtypes.bfloat16)
        if 2 * j + 1 < K:
            wt[64:128, j * 64:(j + 1) * 64] = (
                W[2 * j + 1].astype(ml_dtypes.bfloat16))

    cio = np.tile(np.arange(128, dtype=np.float32), (128, 1)).astype(
        ml_dtypes.bfloat16)
    gb = np.stack([np.asarray(gamma, np.float32),
                   np.asarray(beta, np.float32)])

    in_maps = []
    for c in range(n_cores):
        p = per_core[c]
        ftab = np.zeros((128, S), dtype=ml_dtypes.bfloat16)
        real = p["feat_row"] >= 0
        rows = p["feat_row"][real]
        halves = p["half"][real]
        cols = np.nonzero(real)[0]
        lo = halves == 0
        ftab[0:64, cols[lo]] = feats_bf[rows[lo]].T
        ftab[64:128, cols[~lo]] = feats_bf[rows[~lo]].T
        ridx = np.zeros((128, ntiles), dtype=np.float32)
        ridx[:, :] = p["ridx"].reshape(ntiles, 128).T
        pidx = np.arange(128, dtype=np.float32).reshape(128, 1)
        in_maps.append(dict(ftab=ftab, wt=wt, ridx=ridx, cio=cio, gb=gb,
                            pidx=pidx))

    nc = _build(plan, n_out, n_cores)
    return nc, in_maps, plan


def _unpack_y(res, plan, n_cores):
    NBLK = plan["NBLK"]
    row_order = plan["row_order"]       # [n_cores, NBLK, 128]
    n_out = row_order.max() + 1
    y_full = np.zeros((int(n_out), 64), np.float32)
    for c in range(n_cores):
        yc = np.asarray(res.results[c]["y"]).reshape(128, NBLK, 65)[:, :, :64]
        ro = row_order[c]               # [NBLK, 128]
        valid = ro >= 0
        y_full[ro[valid]] = yc.transpose(1, 0, 2)[valid].astype(np.float32)
    return y_full


def kernel(feats, W, gamma, beta, in_map, out_map, n_out):
    from concourse.bass_utils import run_bass_kernel_spmd

    n_cores = 8
    nc, in_maps, plan = _prepare(
        feats, W, gamma, beta, in_map, out_map, int(n_out), n_cores)
    res = run_bass_kernel_spmd(nc, in_maps, list(range(n_cores)))
    return _unpack_y(res, plan, n_cores)
